# revision 19
# baseline (speedup 1.0000x reference)
import os
import sys

import numpy as np

sys.path.insert(0, "/opt/trn_rl_repo")

import ml_dtypes  # noqa: E402

LAST_EXEC_NS = None

B, T, V = 64, 128, 512
H = 512
A = 64
C = 500
EPS = 1e-5
NSH = 4          # batch shards
BL = B // NSH    # 16 rows per core
NG = 4 * H       # 2048 gates per direction
bf16 = ml_dtypes.bfloat16

_NC_CACHE = {}


def _ceng():
    import concourse.bass as bass  # noqa: F401
    import concourse.tile as tile
    from concourse import bacc, mybir, bass_isa

    return bacc, mybir, bass_isa, tile


# ---------------------------------------------------------------------------
# recurrence: 128 steps of a LayerNorm-LSTM direction, all fp32.
#   whh_sb [128, (k4)(tu16)(128)] f32   lhsT tiles, gate g = tu*128 + p
#   xt_d   DRAM [128, (t T)(tu16)(b16)] f32  (X~ = LN(xg)*g + b(+b_hh))
#   hs_sb  [128, (t T)(uc4)(b16)] f32   unit u = uc*128 + p
def _emit_recurrence(nc, mybir, bass_isa, tc, sp, xp, pp, whh_sb, xt_d, hs_sb,
                     consts, off_ghh, off_gc, off_bc, off_eps, nT):
    AF = mybir.ActivationFunctionType
    ALU = mybir.AluOpType
    f32 = mybir.dt.float32

    eps_ap = consts[:, off_eps:off_eps + 1]
    ghh = consts[:, off_ghh:off_ghh + 16]
    gc = consts[:, off_gc:off_gc + 4]
    bc = consts[:, off_bc:off_bc + 4]

    hz = sp.tile([128, 64], f32, tag="hz")
    nc.vector.memset(hz[:], 0.0)
    cA = sp.tile([128, 64], f32, tag="cA")
    cB = sp.tile([128, 64], f32, tag="cB")
    nc.vector.memset(cA[:], 0.0)

    NPF = 6
    xts = {}

    def prefetch(t):
        if t < nT:
            xt = xp.tile([128, 256], f32, tag="xtin")
            nc.sync.dma_start(xt[:], xt_d[:, t * 256:(t + 1) * 256])
            xts[t] = xt

    for t in range(NPF):
        prefetch(t)

    for t in range(nT):
        z = pp.tile([128, 256], f32, tag="z")
        for tu in range(16):
            for k in range(4):
                rhs = (hs_sb[:, (t - 1) * 64 + k * 16:(t - 1) * 64 + k * 16 + 16]
                       if t > 0 else hz[:, k * 16:k * 16 + 16])
                nc.tensor.matmul(
                    z[:, tu * 16:(tu + 1) * 16],
                    whh_sb[:, (k * 16 + tu) * 128:(k * 16 + tu + 1) * 128],
                    rhs, start=(k == 0), stop=(k == 3))
        # ---- LN(hh) rstd: mean==0 by weight fold
        sq = sp.tile([128, 256], f32, tag="sq")
        nc.scalar.square(sq[:], z[:])
        s2 = sp.tile([128, 16], f32, tag="s2")
        nc.vector.tensor_reduce(
            s2[:], sq[:].rearrange("p (tu b) -> p tu b", tu=16).transpose([0, 2, 1]),
            mybir.AxisListType.X, ALU.add)
        S2 = sp.tile([128, 16], f32, tag="S2")
        nc.gpsimd.partition_all_reduce(S2[:], s2[:], channels=128,
                                       reduce_op=bass_isa.ReduceOp.add)
        sd = sp.tile([128, 16], f32, tag="sd")
        nc.scalar.activation(sd[:], S2[:], AF.Sqrt, bias=eps_ap, scale=1.0 / NG)
        rstd = sp.tile([128, 16], f32, tag="rstd")
        nc.vector.reciprocal(rstd[:], sd[:])
        # ---- gates = z*rstd_b*gamma_tu + X~_t
        zn = sp.tile([128, 256], f32, tag="zn")
        nc.vector.tensor_tensor(
            zn[:].rearrange("p (tu b) -> p tu b", tu=16),
            z[:].rearrange("p (tu b) -> p tu b", tu=16),
            rstd[:].unsqueeze(1).broadcast_to([128, 16, 16]), ALU.mult)
        zg = sp.tile([128, 256], f32, tag="zg")
        nc.vector.tensor_tensor(
            zg[:].rearrange("p (tu b) -> p tu b", tu=16),
            zn[:].rearrange("p (tu b) -> p tu b", tu=16),
            ghh.unsqueeze(2).broadcast_to([128, 16, 16]), ALU.mult)
        gp = sp.tile([128, 256], f32, tag="gp")
        nc.vector.tensor_tensor(gp[:], zg[:], xts.pop(t)[:], ALU.add)
        prefetch(t + NPF)
        gs = sp.tile([128, 256], f32, tag="gs")
        nc.scalar.activation(gs[:, 0:128], gp[:, 0:128], AF.Sigmoid)
        nc.scalar.activation(gs[:, 128:192], gp[:, 128:192], AF.Tanh)
        nc.scalar.activation(gs[:, 192:256], gp[:, 192:256], AF.Sigmoid)
        # ---- c = f*c + i*g
        c_src = cA if t % 2 == 0 else cB
        c_dst = cB if t % 2 == 0 else cA
        t1 = sp.tile([128, 64], f32, tag="t1")
        nc.vector.tensor_tensor(t1[:], gs[:, 64:128], c_src[:], ALU.mult)
        t2 = sp.tile([128, 64], f32, tag="t2")
        nc.vector.tensor_tensor(t2[:], gs[:, 0:64], gs[:, 128:192], ALU.mult)
        nc.vector.tensor_tensor(c_dst[:], t1[:], t2[:], ALU.add)
        # ---- LN(c)
        csq = sp.tile([128, 64], f32, tag="csq")
        nc.scalar.square(csq[:], c_dst[:])
        cs = sp.tile([128, 16], f32, tag="cs")
        nc.vector.tensor_reduce(
            cs[:], c_dst[:].rearrange("p (uc b) -> p uc b", uc=4).transpose([0, 2, 1]),
            mybir.AxisListType.X, ALU.add)
        cs2 = sp.tile([128, 16], f32, tag="cs2")
        nc.vector.tensor_reduce(
            cs2[:], csq[:].rearrange("p (uc b) -> p uc b", uc=4).transpose([0, 2, 1]),
            mybir.AxisListType.X, ALU.add)
        CS = sp.tile([128, 16], f32, tag="CS")
        nc.gpsimd.partition_all_reduce(CS[:], cs[:], channels=128,
                                       reduce_op=bass_isa.ReduceOp.add)
        CS2 = sp.tile([128, 16], f32, tag="CS2")
        nc.gpsimd.partition_all_reduce(CS2[:], cs2[:], channels=128,
                                       reduce_op=bass_isa.ReduceOp.add)
        mu = sp.tile([128, 16], f32, tag="mu")
        nc.scalar.mul(mu[:], CS[:], 1.0 / H)
        mu2 = sp.tile([128, 16], f32, tag="mu2")
        nc.scalar.square(mu2[:], mu[:])
        ex2 = sp.tile([128, 16], f32, tag="ex2")
        nc.scalar.mul(ex2[:], CS2[:], 1.0 / H)
        var = sp.tile([128, 16], f32, tag="var")
        nc.vector.tensor_sub(var[:], ex2[:], mu2[:])
        sdc = sp.tile([128, 16], f32, tag="sdc")
        nc.scalar.activation(sdc[:], var[:], AF.Sqrt, bias=eps_ap)
        rc = sp.tile([128, 16], f32, tag="rc")
        nc.vector.reciprocal(rc[:], sdc[:])
        # ---- h = sigmoid(o) * tanh((c-mu)*rc*gc + bc)
        cvw = lambda ap: ap.rearrange("p (uc b) -> p uc b", uc=4)
        cc = sp.tile([128, 64], f32, tag="cc")
        nc.vector.tensor_tensor(cvw(cc[:]), cvw(c_dst[:]),
                                mu[:].unsqueeze(1).broadcast_to([128, 4, 16]),
                                ALU.subtract)
        cn = sp.tile([128, 64], f32, tag="cn")
        nc.vector.tensor_tensor(cvw(cn[:]), cvw(cc[:]),
                                rc[:].unsqueeze(1).broadcast_to([128, 4, 16]),
                                ALU.mult)
        cg = sp.tile([128, 64], f32, tag="cg")
        nc.vector.tensor_tensor(cvw(cg[:]), cvw(cn[:]),
                                gc.unsqueeze(2).broadcast_to([128, 4, 16]),
                                ALU.mult)
        cgb = sp.tile([128, 64], f32, tag="cgb")
        nc.vector.tensor_tensor(cvw(cgb[:]), cvw(cg[:]),
                                bc.unsqueeze(2).broadcast_to([128, 4, 16]),
                                ALU.add)
        th = sp.tile([128, 64], f32, tag="th")
        nc.scalar.activation(th[:], cgb[:], AF.Tanh)
        nc.vector.tensor_tensor(hs_sb[:, t * 64:(t + 1) * 64],
                                gs[:, 192:256], th[:], ALU.mult)


# ---------------------------------------------------------------------------
# gram pass: Sz2 per row = x.(G x) + v0.x (+c0 via bias col) -> rstd_rows
# G streamed from DRAM. acc is a [128, 2048] f32 staging tile.
def _emit_gram(nc, mybir, bass_isa, sp, wp, pq, g_d, x_sb, acc, rstd_rows,
               consts, nq, nk, has_v0, bias_col):
    AF = mybir.ActivationFunctionType
    ALU = mybir.AluOpType
    f32 = mybir.dt.float32
    for qc in range(nq):
        gts = []
        for k in range(nk):
            gt = wp.tile([128, 128], f32, tag=f"gt{k % 4}")
            nc.sync.dma_start(
                gt[:], g_d[k * 128:(k + 1) * 128, qc * 128:(qc + 1) * 128])
            gts.append(gt)
        for rb in range(8):
            q = pq.tile([128, 256], f32, tag="q")
            for k in range(nk):
                nc.tensor.matmul(
                    q[:], gts[k][:],
                    x_sb[:, k * 2048 + rb * 256:k * 2048 + (rb + 1) * 256],
                    start=(k == 0), stop=(k == nk - 1))
            xq = sp.tile([128, 256], f32, tag="xq")
            if has_v0:
                nc.vector.scalar_tensor_tensor(
                    xq[:], q[:], consts[:, qc:qc + 1],
                    x_sb[:, qc * 2048 + rb * 256:qc * 2048 + (rb + 1) * 256],
                    ALU.add, ALU.mult)
            else:
                nc.vector.tensor_tensor(
                    xq[:], q[:],
                    x_sb[:, qc * 2048 + rb * 256:qc * 2048 + (rb + 1) * 256],
                    ALU.mult)
            if qc == 0:
                nc.vector.tensor_copy(acc[:, rb * 256:(rb + 1) * 256], xq[:])
            else:
                eng = nc.vector if rb % 2 else nc.gpsimd
                eng.tensor_tensor(acc[:, rb * 256:(rb + 1) * 256],
                                  acc[:, rb * 256:(rb + 1) * 256],
                                  xq[:], ALU.add)
    nc.gpsimd.partition_all_reduce(rstd_rows[:], acc[:], channels=128,
                                   reduce_op=bass_isa.ReduceOp.add)
    nc.scalar.activation(acc[:], rstd_rows[:], AF.Sqrt,
                         bias=consts[:, bias_col:bias_col + 1], scale=1.0 / NG)
    nc.vector.reciprocal(rstd_rows[:], acc[:])


# ---------------------------------------------------------------------------
# input projection: X~ = LN(x @ W~.T)*g + b -> xt_d (DRAM), fp32.
# weights streamed from DRAM; x resident in SBUF.
def _emit_inproj(nc, mybir, bass_isa, tc, sp, wp, stp, pp, w_d, x_sb, xt_d,
                 consts, nk, off_g, off_b, off_u, rstd_rows):
    AF = mybir.ActivationFunctionType
    ALU = mybir.AluOpType
    f32 = mybir.dt.float32

    NR = T * BL  # 2048 rows (t,b)
    nrb = NR // 256
    use_gram = rstd_rows is not None

    for rb in range(nrb):
        stage = stp.tile([128, 4096], f32, tag="stage")
        stv = stage[:].rearrange("p (t tu b) -> p t tu b", t=16, tu=16)
        if not use_gram:
            zkeep = sp.tile([128, 16 * 256], f32, tag="pzk", bufs=2)
            accs = []
        for tu in range(16):
            wts = []
            for k in range(nk):
                wt = wp.tile([128, 128], f32, tag=f"wt{k % 4}")
                nc.sync.dma_start(
                    wt[:], w_d[k * 128:(k + 1) * 128, tu * 128:(tu + 1) * 128])
                wts.append(wt)
            z = pp.tile([128, 256], f32, tag="pz")
            for k in range(nk):
                nc.tensor.matmul(
                    z[:], wts[k][:],
                    x_sb[:, k * 2048 + rb * 256:k * 2048 + (rb + 1) * 256],
                    start=(k == 0), stop=(k == nk - 1))
            if use_gram:
                zr = sp.tile([128, 256], f32, tag="pzr")
                if off_u is not None:
                    nc.vector.scalar_tensor_tensor(
                        zr[:], z[:], consts[:, off_u + tu:off_u + tu + 1],
                        rstd_rows[:, rb * 256:(rb + 1) * 256], ALU.add, ALU.mult)
                else:
                    nc.vector.tensor_tensor(
                        zr[:], z[:], rstd_rows[:, rb * 256:(rb + 1) * 256],
                        ALU.mult)
                nc.scalar.activation(
                    stv[:, :, tu, :], zr[:].rearrange("p (t b) -> p t b", t=16),
                    AF.Identity, bias=consts[:, off_b + tu:off_b + tu + 1],
                    scale=consts[:, off_g + tu:off_g + tu + 1])
            else:
                nc.scalar.copy(zkeep[:, tu * 256:(tu + 1) * 256], z[:])
                sqt = sp.tile([128, 256], f32, tag=f"psq{tu % 4}")
                nc.scalar.square(sqt[:], z[:])
                if tu < 4:
                    acc = sp.tile([128, 256], f32, tag=f"pacc{tu}", bufs=2)
                    nc.vector.tensor_copy(acc[:], sqt[:])
                    accs.append(acc)
                else:
                    eng = nc.vector if tu % 2 else nc.gpsimd
                    eng.tensor_tensor(accs[tu % 4][:], accs[tu % 4][:], sqt[:],
                                      ALU.add)
        if not use_gram:
            nc.vector.tensor_tensor(accs[0][:], accs[0][:], accs[1][:], ALU.add)
            nc.gpsimd.tensor_tensor(accs[2][:], accs[2][:], accs[3][:], ALU.add)
            nc.vector.tensor_tensor(accs[0][:], accs[0][:], accs[2][:], ALU.add)
            S = sp.tile([128, 256], f32, tag="pS")
            nc.gpsimd.partition_all_reduce(S[:], accs[0][:], channels=128,
                                           reduce_op=bass_isa.ReduceOp.add)
            sd = sp.tile([128, 256], f32, tag="psd")
            nc.scalar.activation(sd[:], S[:], AF.Sqrt,
                                 bias=consts[:, off_u:off_u + 1], scale=1.0 / NG)
            rst = sp.tile([128, 256], f32, tag="prst")
            nc.vector.reciprocal(rst[:], sd[:])
            for tu in range(16):
                zr = sp.tile([128, 256], f32, tag="pzr")
                nc.vector.tensor_tensor(zr[:], zkeep[:, tu * 256:(tu + 1) * 256],
                                        rst[:], ALU.mult)
                nc.scalar.activation(
                    stv[:, :, tu, :], zr[:].rearrange("p (t b) -> p t b", t=16),
                    AF.Identity, bias=consts[:, off_b + tu:off_b + tu + 1],
                    scale=consts[:, off_g + tu:off_g + tu + 1])
        nc.sync.dma_start(xt_d[:, rb * 4096:(rb + 1) * 4096], stage[:])


def _build_A():
    if "A" in _NC_CACHE:
        return _NC_CACHE["A"]
    bacc, mybir, bass_isa, tile = _ceng()
    from contextlib import ExitStack
    AF = mybir.ActivationFunctionType
    ALU = mybir.AluOpType
    f32 = mybir.dt.float32

    nc = bacc.Bacc("TRN2", target_bir_lowering=False, debug=False)
    xT = nc.dram_tensor("xT", [V, T * BL], f32, kind="ExternalInput")
    w0 = nc.dram_tensor("w0", [V, NG], f32, kind="ExternalInput")
    gq = nc.dram_tensor("gq", [V, V], f32, kind="ExternalInput")
    whh = nc.dram_tensor("whh", [H, NG], f32, kind="ExternalInput")
    cst = nc.dram_tensor("cst", [128, 80], f32, kind="ExternalInput")
    hs_out = nc.dram_tensor("hs", [128, T * 64], f32, kind="ExternalOutput")
    xt_d = nc.dram_tensor("xt_d", [128, T * 256], f32, kind="Internal")

    with tile.TileContext(nc) as tc:
        with ExitStack() as ctx:
            bp = ctx.enter_context(tc.tile_pool(name="big", bufs=1))
            sp = ctx.enter_context(tc.tile_pool(name="scr", bufs=2))
            wp = ctx.enter_context(tc.tile_pool(name="wst", bufs=3))
            stp = ctx.enter_context(tc.tile_pool(name="stg", bufs=2))
            xp = ctx.enter_context(tc.tile_pool(name="xst", bufs=7))
            pp = ctx.enter_context(tc.tile_pool(name="ps", bufs=2, space="PSUM"))
            pq = ctx.enter_context(tc.tile_pool(name="psq", bufs=4, space="PSUM"))

            x_sb = bp.tile([128, 4 * 2048], f32, tag="x")
            for k in range(4):
                nc.sync.dma_start(x_sb[:, k * 2048:(k + 1) * 2048],
                                  xT[k * 128:(k + 1) * 128, :])
            whh_sb = bp.tile([128, 4 * 2048], f32, tag="whh")
            for k in range(4):
                nc.sync.dma_start(whh_sb[:, k * 2048:(k + 1) * 2048],
                                  whh[k * 128:(k + 1) * 128, :])
            consts = bp.tile([128, 80], f32, tag="cst")
            nc.sync.dma_start(consts[:], cst[:])
            hs_sb = bp.tile([128, T * 64], f32, tag="hs")
            rstd_rows = bp.tile([128, 2048], f32, tag="rr")
            acc = bp.tile([128, 2048], f32, tag="gacc")

            _emit_gram(nc, mybir, bass_isa, sp, wp, pq, gq, x_sb, acc,
                       rstd_rows, consts, 4, 4, True, 4)
            _emit_inproj(nc, mybir, bass_isa, tc, sp, wp, stp, pp, w0, x_sb,
                         xt_d, consts, 4, 6, 22, 38, rstd_rows)
            _emit_recurrence(nc, mybir, bass_isa, tc, sp, xp, pp, whh_sb, xt_d,
                             hs_sb, consts, 54, 70, 74, 5, T)
            nc.sync.dma_start(hs_out[:], hs_sb[:])

    nc.compile()
    _NC_CACHE["A"] = nc
    return nc


def _build_B():
    if "B" in _NC_CACHE:
        return _NC_CACHE["B"]
    bacc, mybir, bass_isa, tile = _ceng()
    from contextlib import ExitStack
    f32 = mybir.dt.float32

    nc = bacc.Bacc("TRN2", target_bir_lowering=False, debug=False)
    x1T = nc.dram_tensor("x1T", [2 * H, T * BL], f32, kind="ExternalInput")
    w1 = nc.dram_tensor("w1", [2 * H, NG], f32, kind="ExternalInput")
    gq1 = nc.dram_tensor("gq1", [2 * H, 2 * H], f32, kind="ExternalInput")
    whh = nc.dram_tensor("whh", [H, NG], f32, kind="ExternalInput")
    cst = nc.dram_tensor("cst", [128, 60], f32, kind="ExternalInput")
    hs_out = nc.dram_tensor("hs", [128, T * 64], f32, kind="ExternalOutput")
    xt_d = nc.dram_tensor("xt_d", [128, T * 256], f32, kind="Internal")

    with tile.TileContext(nc) as tc:
        with ExitStack() as ctx:
            bp = ctx.enter_context(tc.tile_pool(name="big", bufs=1))
            sp = ctx.enter_context(tc.tile_pool(name="scr", bufs=2))
            wp = ctx.enter_context(tc.tile_pool(name="wst", bufs=3))
            stp = ctx.enter_context(tc.tile_pool(name="stg", bufs=1))
            xp = ctx.enter_context(tc.tile_pool(name="xst", bufs=7))
            pp = ctx.enter_context(tc.tile_pool(name="ps", bufs=2, space="PSUM"))
            pq = ctx.enter_context(tc.tile_pool(name="psq", bufs=4, space="PSUM"))

            x_sb = bp.tile([128, 8 * 2048], f32, tag="x")
            for k in range(8):
                nc.sync.dma_start(x_sb[:, k * 2048:(k + 1) * 2048],
                                  x1T[k * 128:(k + 1) * 128, :])
            whh_sb = bp.tile([128, 4 * 2048], f32, tag="whh")
            for k in range(4):
                nc.sync.dma_start(whh_sb[:, k * 2048:(k + 1) * 2048],
                                  whh[k * 128:(k + 1) * 128, :])
            consts = bp.tile([128, 60], f32, tag="cst")
            nc.sync.dma_start(consts[:], cst[:])
            hs_sb = bp.tile([128, T * 64], f32, tag="hs")
            rstd_rows = bp.tile([128, 2048], f32, tag="rr")
            acc = bp.tile([128, 2048], f32, tag="gacc")

            # B consts: 0:16 g1, 16:32 b1, 32:48 ghh, 48:52 gcc, 52:56 bcc, 56 eps
            _emit_gram(nc, mybir, bass_isa, sp, wp, pq, gq1, x_sb, acc,
                       rstd_rows, consts, 8, 8, False, 56)
            _emit_inproj(nc, mybir, bass_isa, tc, sp, wp, stp, pp, w1, x_sb,
                         xt_d, consts, 8, 0, 16, None, rstd_rows)
            _emit_recurrence(nc, mybir, bass_isa, tc, sp, xp, pp, whh_sb, xt_d,
                             hs_sb, consts, 32, 48, 52, 56, T)
            nc.sync.dma_start(hs_out[:], hs_sb[:])

    nc.compile()
    _NC_CACHE["B"] = nc
    return nc


def _build_C():
    if "C" in _NC_CACHE:
        return _NC_CACHE["C"]
    bacc, mybir, bass_isa, tile = _ceng()
    from contextlib import ExitStack
    AF = mybir.ActivationFunctionType
    ALU = mybir.AluOpType
    f32 = mybir.dt.float32
    dbf = mybir.dt.bfloat16
    BC = B // 8  # 8 batch rows per core

    nc = bacc.Bacc("TRN2", target_bir_lowering=False, debug=False)
    hsb = nc.dram_tensor("hsb", [2 * H, T * BC], dbf, kind="ExternalInput")
    wom = nc.dram_tensor("wom", [2 * H, A], dbf, kind="ExternalInput")
    wp = nc.dram_tensor("wp", [2 * H, 512], dbf, kind="ExternalInput")
    cs2 = nc.dram_tensor("cs2", [A, 3], f32, kind="ExternalInput")  # bom, uom
    w0p = nc.dram_tensor("w0p", [1, 512], f32, kind="ExternalInput")
    logit = nc.dram_tensor("logit", [BC, 512], f32, kind="ExternalOutput")

    NRC = T * BC  # 1024 cols (t,b)
    with tile.TileContext(nc) as tc:
        with ExitStack() as ctx:
            bp = ctx.enter_context(tc.tile_pool(name="big", bufs=1))
            sp = ctx.enter_context(tc.tile_pool(name="scr", bufs=2))
            pp = ctx.enter_context(tc.tile_pool(name="ps", bufs=4, space="PSUM"))

            h_sb = bp.tile([128, 8 * NRC], dbf, tag="h")
            for k in range(8):
                nc.sync.dma_start(h_sb[:, k * NRC:(k + 1) * NRC],
                                  hsb[k * 128:(k + 1) * 128, :])
            wom_sb = bp.tile([128, 8 * A], dbf, tag="wom")
            for k in range(8):
                nc.sync.dma_start(wom_sb[:, k * A:(k + 1) * A],
                                  wom[k * 128:(k + 1) * 128, :])
            wp_sb = bp.tile([128, 8 * 512], dbf, tag="wp")
            for k in range(8):
                nc.sync.dma_start(wp_sb[:, k * 512:(k + 1) * 512],
                                  wp[k * 128:(k + 1) * 128, :])
            c2 = bp.tile([A, 3], f32, tag="c2")
            nc.sync.dma_start(c2[:], cs2[:])
            w0_sb = bp.tile([1, 512], f32, tag="w0p")
            nc.sync.dma_start(w0_sb[:], w0p[:])

            # v.T = tanh(wom'.T @ x2) [A, (t,b)]
            vsb = bp.tile([A, NRC], f32, tag="v")
            for nb in range(2):
                vps = pp.tile([A, 512], f32, tag="vps")
                for k in range(8):
                    nc.tensor.matmul(vps[:],
                                     wom_sb[:, k * A:(k + 1) * A],
                                     h_sb[:, k * NRC + nb * 512:k * NRC + (nb + 1) * 512],
                                     start=(k == 0), stop=(k == 7))
                nc.scalar.activation(vsb[:, nb * 512:(nb + 1) * 512], vps[:],
                                     AF.Tanh, bias=c2[:, 0:1])
            # s = u.v  -> softmax over t per b
            ss = sp.tile([A, NRC], f32, tag="ss")
            nc.vector.tensor_scalar_mul(ss[:], vsb[:], c2[:, 1:2])
            S = sp.tile([A, NRC], f32, tag="S")
            nc.gpsimd.partition_all_reduce(S[:], ss[:], channels=A,
                                           reduce_op=bass_isa.ReduceOp.add)
            s0 = S[0:1, :]
            mx = sp.tile([1, BC], f32, tag="mx")
            nc.vector.tensor_reduce(
                mx[:], s0.rearrange("p (t b) -> p t b", t=T).transpose([0, 2, 1]),
                mybir.AxisListType.X, ALU.max)
            es = sp.tile([1, NRC], f32, tag="es")
            nc.vector.tensor_tensor(
                es[:].rearrange("p (t b) -> p t b", t=T),
                s0.rearrange("p (t b) -> p t b", t=T),
                mx[:].unsqueeze(1).broadcast_to([1, T, BC]), ALU.subtract)
            ee = sp.tile([1, NRC], f32, tag="ee")
            nc.scalar.activation(ee[:], es[:], AF.Exp)
            Z = sp.tile([1, BC], f32, tag="Z")
            nc.vector.tensor_reduce(
                Z[:], ee[:].rearrange("p (t b) -> p t b", t=T).transpose([0, 2, 1]),
                mybir.AxisListType.X, ALU.add)
            rz = sp.tile([1, BC], f32, tag="rz")
            nc.vector.reciprocal(rz[:], Z[:])
            al = sp.tile([1, NRC], f32, tag="al")
            nc.vector.tensor_tensor(
                al[:].rearrange("p (t b) -> p t b", t=T),
                ee[:].rearrange("p (t b) -> p t b", t=T),
                rz[:].unsqueeze(1).broadcast_to([1, T, BC]), ALU.mult)
            alb = sp.tile([128, NRC], f32, tag="alb")
            nc.gpsimd.partition_broadcast(alb[:], al[:], channels=128)
            # pooled[ch, b] = sum_t alpha * h
            pooled = sp.tile([128, 8 * BC], f32, tag="pool")
            pm = sp.tile([128, NRC], f32, tag="pm")
            for k in range(8):
                nc.vector.tensor_tensor(pm[:], h_sb[:, k * NRC:(k + 1) * NRC],
                                        alb[:], ALU.mult)
                nc.vector.tensor_reduce(
                    pooled[:, k * BC:(k + 1) * BC],
                    pm[:].rearrange("p (t b) -> p t b", t=T).transpose([0, 2, 1]),
                    mybir.AxisListType.X, ALU.add)
            pooled_bf = sp.tile([128, 8 * BC], dbf, tag="poolb")
            nc.scalar.copy(pooled_bf[:], pooled[:])
            # logits = pooled.T @ wp' + w0p
            lps = pp.tile([BC, 512], f32, tag="lps")
            for k in range(8):
                nc.tensor.matmul(lps[:], pooled_bf[:, k * BC:(k + 1) * BC],
                                 wp_sb[:, k * 512:(k + 1) * 512],
                                 start=(k == 0), stop=(k == 7))
            w0b = sp.tile([BC, 512], f32, tag="w0b")
            nc.gpsimd.partition_broadcast(w0b[:], w0_sb[:], channels=BC)
            lg = sp.tile([BC, 512], f32, tag="lg")
            nc.vector.tensor_tensor(lg[:], lps[:], w0b[:], ALU.add)
            nc.sync.dma_start(logit[:], lg[:])

    nc.compile()
    _NC_CACHE["C"] = nc
    return nc


_HOOKED = False


def _install_profile_hook():
    """antenv.axon_hooks is absent in this image; shim it so trace=True works."""
    global _HOOKED
    if _HOOKED:
        return
    _HOOKED = True
    try:
        import types
        import trn_agent_boot.trn_boot as tb

        hook = tb._ntff_profile_via_ctypes("/opt/axon/libaxon_pjrt.so")
        if hook is not None:
            mod = types.ModuleType("antenv.axon_hooks")
            mod.get_axon_ntff_profile_hook = lambda: hook
            mod.set_axon_ntff_profile_hook = lambda h: None
            sys.modules["antenv.axon_hooks"] = mod
    except Exception:
        pass


def _run(nc, in_maps):
    global LAST_EXEC_NS
    from concourse import bass_utils

    _install_profile_hook()
    want_trace = os.environ.get("KERNEL_TRACE", "1") == "1"
    if want_trace:
        try:
            res = bass_utils.run_bass_kernel_spmd(
                nc, in_maps, core_ids=list(range(len(in_maps))), trace=True)
            if res.exec_time_ns is not None:
                LAST_EXEC_NS = (LAST_EXEC_NS or 0) + res.exec_time_ns
            return res
        except Exception:
            pass
    res = bass_utils.run_bass_kernel_spmd(
        nc, in_maps, core_ids=list(range(len(in_maps))))
    if res.exec_time_ns is not None:
        LAST_EXEC_NS = (LAST_EXEC_NS or 0) + res.exec_time_ns
    return res


def _pack16(vec):
    """[2048] gate vector -> [128,16] (p, tu) f32."""
    return np.ascontiguousarray(vec.reshape(16, 128).T, np.float32)


def _pack4(vec):
    return np.ascontiguousarray(vec.reshape(4, 128).T, np.float32)


def kernel(**inputs):
    global LAST_EXEC_NS
    LAST_EXEC_NS = None
    x = np.asarray(inputs["batch_embedded"], np.float32)  # [B,T,V]
    bn1_g = np.asarray(inputs["bn1_g"], np.float32)
    bn1_b = np.asarray(inputs["bn1_b"], np.float32)
    bn2_g = np.asarray(inputs["bn2_g"], np.float32)
    bn2_b = np.asarray(inputs["bn2_b"], np.float32)
    w_ih0 = np.asarray(inputs["w_ih0"], np.float32)
    w_hh0 = np.asarray(inputs["w_hh0"], np.float32)
    lng0 = np.asarray(inputs["lng0"], np.float32)
    lnb0 = np.asarray(inputs["lnb0"], np.float32)
    w_ih1 = np.asarray(inputs["w_ih1"], np.float32)
    w_hh1 = np.asarray(inputs["w_hh1"], np.float32)
    lng1 = np.asarray(inputs["lng1"], np.float32)
    lnb1 = np.asarray(inputs["lnb1"], np.float32)
    w_omega = np.asarray(inputs["w_omega"], np.float32)
    b_omega = np.asarray(inputs["b_omega"], np.float32)
    u_omega = np.asarray(inputs["u_omega"], np.float32)
    W = np.asarray(inputs["W"], np.float32)

    # ---- host: BN1 batch stats folded into layer-0 input weights
    m1 = x.mean(axis=(0, 1))
    v1 = x.var(axis=(0, 1))
    a1 = bn1_g / np.sqrt(v1 + EPS)
    b1 = bn1_b - m1 * a1

    ncA = _build_A()
    in_A = []
    for c in range(8):
        d, s = (0, c) if c < 4 else (1, c - 4)
        xe = x[s * BL:(s + 1) * BL]            # [16,T,V]
        xt = xe.transpose(2, 1, 0)             # [V,T,B]
        if d == 1:
            xt = xt[:, ::-1, :]
        xt = np.ascontiguousarray(xt.reshape(V, T * BL), np.float32)

        W0p = w_ih0[d] * a1[None, :]           # [2048, 512]
        u0 = w_ih0[d] @ b1                     # [2048]
        m0 = W0p.mean(axis=0)
        W0pp = W0p - m0[None, :]
        u0t = u0 - u0.mean()
        Gq = (W0pp.T @ W0pp).astype(np.float32)
        v0 = 2.0 * (u0t @ W0pp)
        c0 = float((u0t ** 2).sum())

        whh = w_hh0[d]                         # [2048, 512]
        whhpp = whh - whh.mean(axis=0)[None, :]

        cstv = np.zeros((128, 80), np.float32)
        cstv[:, 0:4] = _pack4(v0)
        cstv[:, 4] = EPS + c0 / NG
        cstv[:, 5] = EPS
        cstv[:, 6:22] = _pack16(lng0[d][:NG])
        cstv[:, 22:38] = _pack16(lnb0[d][:NG] + lnb0[d][NG:2 * NG])
        cstv[:, 38:54] = _pack16(u0t)
        cstv[:, 54:70] = _pack16(lng0[d][NG:2 * NG])
        cstv[:, 70:74] = _pack4(lng0[d][2 * NG:])
        cstv[:, 74:78] = _pack4(lnb0[d][2 * NG:])
        in_A.append({
            "xT": xt,
            "w0": np.ascontiguousarray(W0pp.T, np.float32),
            "gq": np.ascontiguousarray(Gq, np.float32),
            "whh": np.ascontiguousarray(whhpp.T, np.float32),
            "cst": cstv,
        })
    resA = _run(ncA, in_A)

    # hs [128, T*64] -> [T, b, u] with u = uc*128+p
    def unpack_hs(hsv, rev):
        a = np.asarray(hsv).reshape(128, T, 4, BL)
        a = a.transpose(1, 3, 2, 0).reshape(T, BL, H)  # u = uc*128+p
        return a[::-1] if rev else a

    hs0 = [unpack_hs(resA.results[c]["hs"], c >= 4) for c in range(8)]

    ncB = _build_B()
    in_B = []
    for c in range(8):
        d, s = (0, c) if c < 4 else (1, c - 4)
        x1 = np.concatenate([hs0[s], hs0[4 + s]], axis=-1)  # [T,16,1024]
        x1t = np.ascontiguousarray(x1.transpose(2, 0, 1))   # [1024,T,B]
        if d == 1:
            x1t = x1t[:, ::-1, :]
        x1t = np.ascontiguousarray(x1t.reshape(2 * H, T * BL), np.float32)

        W1 = w_ih1[d]                          # [2048, 1024]
        W1pp = W1 - W1.mean(axis=0)[None, :]
        Gq1 = (W1pp.T @ W1pp).astype(np.float32)
        whh = w_hh1[d]
        whhpp = whh - whh.mean(axis=0)[None, :]

        cstv = np.zeros((128, 60), np.float32)
        cstv[:, 0:16] = _pack16(lng1[d][:NG])
        cstv[:, 16:32] = _pack16(lnb1[d][:NG] + lnb1[d][NG:2 * NG])
        cstv[:, 32:48] = _pack16(lng1[d][NG:2 * NG])
        cstv[:, 48:52] = _pack4(lng1[d][2 * NG:])
        cstv[:, 52:56] = _pack4(lnb1[d][2 * NG:])
        cstv[:, 56] = EPS
        in_B.append({
            "x1T": x1t,
            "w1": np.ascontiguousarray(W1pp.T, np.float32),
            "gq1": Gq1,
            "whh": np.ascontiguousarray(whhpp.T, np.float32),
            "cst": cstv,
        })
    resB = _run(ncB, in_B)

    hs1 = [unpack_hs(resB.results[c]["hs"], c >= 4) for c in range(8)]
    # x2 [T, B, 2H]
    x2 = np.concatenate(
        [np.concatenate([hs1[s], hs1[4 + s]], axis=-1) for s in range(NSH)],
        axis=1).astype(np.float32)

    # ---- host: BN2 stats
    m2 = x2.mean(axis=(0, 1))
    v2 = x2.var(axis=(0, 1))
    a2 = bn2_g / np.sqrt(v2 + EPS)
    b2 = bn2_b - m2 * a2

    womp = (a2[:, None] * w_omega)             # [1024, 64]
    bomp = b_omega + b2 @ w_omega              # [64]
    wpp = np.zeros((2 * H, 512), np.float32)
    wpp[:, :C + 1] = a2[:, None] * W[:2 * H]
    w0p = np.zeros((1, 512), np.float32)
    w0p[0, :C + 1] = b2 @ W[:2 * H] + W[2 * H]

    cs2 = np.zeros((A, 3), np.float32)
    cs2[:, 0] = bomp
    cs2[:, 1] = u_omega
    ncC = _build_C()
    in_C = []
    BC = B // 8
    for c in range(8):
        xe = x2[:, c * BC:(c + 1) * BC]        # [T, 8, 1024]
        hsb = np.ascontiguousarray(xe.transpose(2, 0, 1).reshape(2 * H, T * BC))
        in_C.append({
            "hsb": hsb.astype(bf16),
            "wom": np.ascontiguousarray(womp).astype(bf16),
            "wp": np.ascontiguousarray(wpp).astype(bf16),
            "cs2": cs2,
            "w0p": w0p,
        })
    resC = _run(ncC, in_C)
    out = np.concatenate(
        [np.asarray(resC.results[c]["logit"])[:, :C + 1] for c in range(8)],
        axis=0).astype(np.float32)
    return out
